# revision 39
# baseline (speedup 1.0000x reference)
"""DigitCapsuleLayer forward (2 routing iterations) on 8 Trainium2 cores.

Pure data-parallel: batch 256 split 32-per-core. Routing math restructured so
u_hat [B,2,6912,16] is never materialized:

  S[b,je]    = sum_m Wf[m,je] * x[m,b]          (m = (n,d) flattened, 55296)
  v1         = squash(0.5*S)
  g[m,b]     = sum_je Wf[m,je] * vtil[je,b]     (vtil = [v1_j0, -v1_j1])
  Delta[n,b] = sum_d g[(n,d),b] * x[(n,d),b]    (block-diag ones matmul)
  c0         = sigmoid(Delta) broadcast over d  (replication matmul)
  y0         = c0 * x
  A[b,je]    = sum_m Wf[m,je] * y0[m,b]
  s2_j0 = A_j0 ; s2_j1 = S_j1 - A_j1            (since c1 = 1-c0)
  v = squash(s2)

Perf structure:
 - phase-1 S and phase-6 A matmuls are 4-tile packed ([128x128]x[128x128]
   with diagonal-block extraction) -> 108 matmuls instead of 432 each,
   cutting PE sequencer time 4x.
 - DMA order: xt/wf slices interleaved (phase-1 streams during DMA),
   then wft (fp8, only needed once the routing pipeline starts).
 - PSUM->SBUF g copies run mostly on the (otherwise idle) GpSimd engine,
   sigmoid on Activation, multiplies on Vector.
"""

import os
os.environ.setdefault("NEURON_RT_RESET_CORES", "1")

import numpy as np
import ml_dtypes

import concourse.bacc as bacc
import concourse.mybir as mybir
import concourse.tile as tile
from concourse.bass_utils import run_bass_kernel_spmd

# Problem constants (hardcoded per harness contract)
B = 256
NCORES = 8
BC = B // NCORES          # 32 batch per core
N = 6912
D = 8
E = 16
J = 2
M = N * D                 # 55296
JE = J * E                # 32
NT = M // 128             # 432 m-tiles
NP = NT // 4              # 108 4-tile packs
NG = NT // 4              # 108 groups of 4 (row-packed g matmuls)
CH = 27                   # 512-col chunks of the [128, 13824] monoliths
FREE = NT * BC            # 13824
EPS = 1e-9

BF16 = mybir.dt.bfloat16
F8 = mybir.dt.float8e4
F32 = mybir.dt.float32

_cached = None


def _build_program():
    nc = bacc.Bacc("TRN2", num_devices=NCORES)

    xt = nc.dram_tensor("xt", [128, FREE], BF16, kind="ExternalInput")
    wf = nc.dram_tensor("wf", [128, FREE], BF16, kind="ExternalInput")
    wft = nc.dram_tensor("wft", [128, NG * 128], F8, kind="ExternalInput")
    sumrep = nc.dram_tensor("sumrep", [128, 128], BF16, kind="ExternalInput")
    vout = nc.dram_tensor("vout", [BC, JE], F32, kind="ExternalOutput")

    SIG = mybir.ActivationFunctionType.Sigmoid
    SQRT = mybir.ActivationFunctionType.Sqrt

    with tile.TileContext(nc) as tc:
        with (
            tc.tile_pool(name="big", bufs=1) as big,
            tc.tile_pool(name="small", bufs=1) as small,
            tc.tile_pool(name="p_gbf", bufs=3) as p_gbf,
            tc.tile_pool(name="p_tch", bufs=4) as p_tch,
            tc.tile_pool(name="p_cbf", bufs=4) as p_cbf,
            tc.tile_pool(name="p_ybf", bufs=5) as p_ybf,
            tc.tile_pool(name="ps_S", bufs=1, space="PSUM") as ps_S,
            tc.tile_pool(name="ps_A", bufs=1, space="PSUM") as ps_A,
            tc.tile_pool(name="ps_g", bufs=3, space="PSUM") as ps_g,
            tc.tile_pool(name="ps_d", bufs=3, space="PSUM") as ps_d,
        ):
            XT = big.tile([128, FREE], BF16, tag="XT")
            WF = big.tile([128, FREE], BF16, tag="WF")
            WFT = big.tile([128, NG * 128], F8, tag="WFT")
            SUMREP = small.tile([128, 128], BF16, tag="SUMREP")
            VTBD = small.tile([128, 128], BF16, tag="VTBD")

            # VTBD zeroed early (off critical path)
            nc.vector.memset(VTBD[:], 0.0)

            # ---- DMA: sumrep first, then interleaved xt/wf slices; wft last ----
            nc.sync.dma_start(SUMREP[:], sumrep[:])
            NSL = 8
            slw = FREE // NSL
            for i in range(NSL):
                nc.sync.dma_start(XT[:, i * slw:(i + 1) * slw], xt[:, i * slw:(i + 1) * slw])
                nc.sync.dma_start(WF[:, i * slw:(i + 1) * slw], wf[:, i * slw:(i + 1) * slw])
            wslw = NG * 128 // NSL
            for i in range(NSL):
                nc.sync.dma_start(WFT[:, i * wslw:(i + 1) * wslw], wft[:, i * wslw:(i + 1) * wslw])

            psSa = ps_S.tile([128, 128], F32, tag="psSa")
            psSb = ps_A.tile([128, 128], F32, tag="psSb")  # reused later as psA

            # PE p-state warmers: tiny matmuls that keep the tensor engine
            # continuously busy so it ramps to (and stays at) full clock.
            # Scratch target borrows a ps_d buffer (pipeline reuses it later,
            # after all warmers are done).
            wtile = ps_d.tile([128, 512], F32, tag="psd")

            def warm(n):
                for _ in range(n):
                    nc.tensor.matmul(
                        wtile[:, 0:64], lhsT=SUMREP[:], rhs=SUMREP[:, 0:64],
                        start=True, stop=True,
                    )
            H = NP // 2
            warm(60)  # bridge the wait for the first xt/wf slices
            for j in range(NP):
                ps = psSa if j < H else psSb
                nc.tensor.matmul(
                    ps[:],
                    lhsT=XT[:, j * 128:(j + 1) * 128],
                    rhs=WF[:, j * 128:(j + 1) * 128],
                    start=(j % H == 0),
                    stop=(j % H == H - 1),
                )
                warm(2)
            warm(60)

            # ---- S extraction: sum diagonal 32x32 blocks of both halves ----
            t01 = small.tile([BC, JE], F32, tag="t01")
            t23 = small.tile([BC, JE], F32, tag="t23")
            S = small.tile([BC, JE], F32, tag="S")
            # first-half extraction runs early (hidden under DMA)
            nc.vector.tensor_copy(t01[:], psSa[0:32, 0:32])
            nc.vector.tensor_add(t01[:], t01[:], psSa[32:64, 32:64])
            nc.vector.tensor_add(t01[:], t01[:], psSa[64:96, 64:96])
            nc.vector.tensor_add(t01[:], t01[:], psSa[96:128, 96:128])
            nc.vector.tensor_copy(t23[:], psSb[0:32, 0:32])
            nc.vector.tensor_add(t23[:], t23[:], psSb[32:64, 32:64])
            nc.vector.tensor_add(t23[:], t23[:], psSb[64:96, 64:96])
            nc.vector.tensor_add(t23[:], t23[:], psSb[96:128, 96:128])
            nc.vector.tensor_add(S[:], t01[:], t23[:])

            # ---- squash #1: v1 = squash(0.5*S); vt = [v1_j0, -v1_j1] ----
            # n2 = |0.5 S|^2 = 0.25*n2s; vt = S * (+-0.125 * n2s * r1 * rq)
            sq = small.tile([BC, JE], F32, tag="sq")
            n2s = small.tile([BC, J], F32, tag="n2s")
            d1 = small.tile([BC, J], F32, tag="d1")
            r1 = small.tile([BC, J], F32, tag="r1")
            q = small.tile([BC, J], F32, tag="q")
            rq = small.tile([BC, J], F32, tag="rq")
            f = small.tile([BC, J], F32, tag="f")
            vt = small.tile([BC, JE], BF16, tag="vt")

            MUL = mybir.AluOpType.mult
            ADD = mybir.AluOpType.add
            nc.vector.tensor_mul(sq[:], S[:], S[:])
            nc.vector.reduce_sum(
                n2s[:], sq.rearrange("p (j e) -> p j e", e=E), axis=mybir.AxisListType.X
            )
            nc.vector.tensor_scalar(d1[:], n2s[:], 0.25, 1.0, MUL, ADD)
            nc.vector.reciprocal(r1[:], d1[:])
            nc.vector.tensor_scalar(q[:], n2s[:], 0.25, EPS, MUL, ADD)
            nc.scalar.activation(q[:], q[:], SQRT)
            nc.vector.reciprocal(rq[:], q[:])
            nc.vector.tensor_mul(f[:], r1[:], rq[:])
            nc.vector.tensor_mul(f[:], f[:], n2s[:])
            nc.vector.tensor_scalar(vt[:, 0:E], S[:, 0:E], f[:, 0:1], 0.125, MUL, MUL)
            nc.vector.tensor_scalar(vt[:, E:JE], S[:, E:JE], f[:, 1:2], -0.125, MUL, MUL)
            # 4 diag-block transposes into VTBD
            for a in range(4):
                nc.vector.transpose(VTBD[32 * a:32 * a + 32, 32 * a:32 * a + 32], vt[:])

            # ---- pipeline: g -> copy -> tch -> Delta -> sigmoid -> y0 -> A ----
            # reuse psSb's bank as the A accumulator (dead after S-extraction;
            # the first s2 matmul has start=True so it overwrites)
            psA = psSb
            tch_l, ybf_l = {}, {}

            psg_l, psd_l, cbf_l = {}, {}, {}

            def g_stage(K):
                psg = ps_g.tile([128, 512], F32, tag="psg")
                for qq in range(4):
                    g_idx = 4 * K + qq
                    nc.tensor.matmul(
                        psg[:, qq * 128:(qq + 1) * 128],
                        lhsT=WFT[:, g_idx * 128:(g_idx + 1) * 128],
                        rhs=VTBD[:],
                        start=True,
                        stop=True,
                    )
                psg_l[K] = psg

            def t_stage(K):
                lo, hi = K * 512, (K + 1) * 512
                tch = p_tch.tile([128, 512], BF16, tag="tch")
                # fused PSUM->SBUF convert + multiply on DVE
                nc.vector.tensor_mul(tch[:], psg_l.pop(K), XT[:, lo:hi])
                tch_l[K] = tch

            def d_stage(K):
                psd = ps_d.tile([128, 512], F32, tag="psd")
                nc.tensor.matmul(
                    psd[:], lhsT=SUMREP[:], rhs=tch_l.pop(K), start=True, stop=True
                )
                psd_l[K] = psd

            def sig_stage(K):
                cbf = p_cbf.tile([128, 512], BF16, tag="cbf")
                nc.scalar.activation(cbf[:], psd_l.pop(K)[:], SIG)
                cbf_l[K] = cbf

            def y_stage(K):
                lo, hi = K * 512, (K + 1) * 512
                ybf = p_ybf.tile([128, 512], BF16, tag="ybf")
                # y0 = c * x: ~3/5 of chunks on the otherwise-idle GpSimd engine
                # (last chunks on DVE so the drain is short)
                eng = nc.gpsimd if (K % 2 == 0 and K < CH - 2) else nc.vector
                eng.tensor_mul(ybf[:], cbf_l.pop(K), XT[:, lo:hi])
                ybf_l[K] = ybf

            def s2_stage(K):
                ybf = ybf_l.pop(K)
                for qq in range(4):
                    j = 4 * K + qq
                    nc.tensor.matmul(
                        psA[:],
                        lhsT=ybf[:, qq * 128:(qq + 1) * 128],
                        rhs=WF[:, j * 128:(j + 1) * 128],
                        start=(j == 0),
                        stop=(j == NP - 1),
                    )

            for i in range(CH + 9):
                if i < CH:
                    g_stage(i)
                if 0 <= i - 2 < CH:
                    t_stage(i - 2)
                if 0 <= i - 4 < CH:
                    d_stage(i - 4)
                if 0 <= i - 5 < CH:
                    sig_stage(i - 5)
                if 0 <= i - 6 < CH:
                    y_stage(i - 6)
                if 0 <= i - 8 < CH:
                    s2_stage(i - 8)

            # ---- A extraction + s2 combine + squash #2 ----
            u01 = small.tile([BC, JE], F32, tag="u01")
            u23 = small.tile([BC, JE], F32, tag="u23")
            A = small.tile([BC, JE], F32, tag="A")
            nc.vector.tensor_copy(u01[:], psA[0:32, 0:32])
            nc.vector.tensor_add(u01[:], u01[:], psA[32:64, 32:64])
            nc.vector.tensor_copy(u23[:], psA[64:96, 64:96])
            nc.vector.tensor_add(u23[:], u23[:], psA[96:128, 96:128])
            nc.vector.tensor_add(A[:], u01[:], u23[:])

            s2 = small.tile([BC, JE], F32, tag="s2")
            sq2 = small.tile([BC, JE], F32, tag="sq2")
            n2b = small.tile([BC, J], F32, tag="n2b")
            d1b = small.tile([BC, J], F32, tag="d1b")
            r1b = small.tile([BC, J], F32, tag="r1b")
            qb = small.tile([BC, J], F32, tag="qb")
            rqb = small.tile([BC, J], F32, tag="rqb")
            fb = small.tile([BC, J], F32, tag="fb")
            v2 = small.tile([BC, JE], F32, tag="v2")

            nc.vector.tensor_copy(s2[:, 0:E], A[:, 0:E])
            nc.vector.tensor_sub(s2[:, E:JE], S[:, E:JE], A[:, E:JE])
            nc.vector.tensor_mul(sq2[:], s2[:], s2[:])
            nc.vector.reduce_sum(
                n2b[:], sq2.rearrange("p (j e) -> p j e", e=E), axis=mybir.AxisListType.X
            )
            nc.vector.tensor_scalar(d1b[:], n2b[:], 1.0, None, ADD)
            nc.vector.reciprocal(r1b[:], d1b[:])
            nc.vector.tensor_scalar(qb[:], n2b[:], EPS, None, ADD)
            nc.scalar.activation(qb[:], qb[:], SQRT)
            nc.vector.reciprocal(rqb[:], qb[:])
            nc.vector.tensor_mul(fb[:], r1b[:], rqb[:])
            nc.vector.tensor_scalar(v2[:, 0:E], s2[:, 0:E], fb[:, 0:1], n2b[:, 0:1], MUL, MUL)
            nc.vector.tensor_scalar(v2[:, E:JE], s2[:, E:JE], fb[:, 1:2], n2b[:, 1:2], MUL, MUL)

            nc.sync.dma_start(vout[:], v2[:])

    nc.compile()
    return nc


def _prep_host(x, W):
    """Build per-core DRAM feeds (identical layouts to prior version)."""
    bf = ml_dtypes.bfloat16
    # Wf[(n,d), (j,e)] = W[j,n,e,d]
    Wf = np.ascontiguousarray(np.transpose(W, (1, 3, 0, 2)).reshape(M, JE))
    wf_feed = np.ascontiguousarray(
        Wf.reshape(NT, 128, JE).transpose(1, 0, 2).reshape(128, FREE)
    ).astype(bf)
    # 4-stacked WfT groups: group g rows 32a+k hold Wf[m=128*(4g+a)+f, k]
    wft_np = np.empty((NG, 128, 128), dtype=np.float32)
    blocks = Wf.reshape(NT, 128, JE)                    # [432, 128, 32]
    for a in range(4):
        wft_np[:, 32 * a:32 * a + 32, :] = blocks[a::4].transpose(0, 2, 1)
    wft_feed = np.ascontiguousarray(
        wft_np.transpose(1, 0, 2).reshape(128, NG * 128)
    ).astype(ml_dtypes.float8_e4m3)

    p = np.arange(128)
    sumrep_np = (p[:, None] // D == p[None, :] // D).astype(bf)

    in_maps = []
    for c in range(NCORES):
        xs = x[c * BC:(c + 1) * BC].reshape(BC, M).T      # [m, b]
        xt_feed = np.ascontiguousarray(
            xs.reshape(NT, 128, BC).transpose(1, 0, 2).reshape(128, FREE)
        ).astype(bf)
        in_maps.append({
            "xt": xt_feed,
            "wf": wf_feed,
            "wft": wft_feed,
            "sumrep": sumrep_np,
        })
    return in_maps


def kernel(x, W):
    global _cached
    x = np.asarray(x, dtype=np.float32)
    W = np.asarray(W, dtype=np.float32)
    if _cached is None:
        _cached = _build_program()
    nc = _cached
    in_maps = _prep_host(x, W)
    res = run_bass_kernel_spmd(nc, in_maps, list(range(NCORES)))
    out = np.concatenate(
        [res.results[c]["vout"].reshape(BC, J, E) for c in range(NCORES)], axis=0
    )
    return out.astype(np.float32)


if __name__ == "__main__":
    import sys
    sys.path.insert(0, "/root/problem")
    import reference as ref
    inputs = ref.setup_inputs()
    expected = np.asarray(ref.reference(**inputs))
    actual = kernel(np.asarray(inputs["x"]), np.asarray(inputs["W"]))
    err = np.abs(actual - expected)
    scale = np.abs(expected).max()
    print("absmax err:", err.max(), "scale:", scale, "rel:", err.max() / scale)
